# revision 11
# baseline (speedup 1.0000x reference)
"""Trainium2 Bass kernel for nn_AttentionBlock (GroupNorm + MHSA + proj residual).

Contract: kernel(**inputs) takes the FULL inputs from setup_inputs() and
returns the FULL output [16, 512, 32, 32]. Internally shards batch over the
8 NeuronCores (2 batches per core), runs one SPMD Bass program, gathers.
"""

import sys
from contextlib import ExitStack

sys.path.insert(0, "/opt/trn_rl_repo")

import numpy as np

import concourse.bass as bass
import concourse.bacc as bacc
import concourse.mybir as mybir
import concourse.tile as tile
from concourse.bass_utils import run_bass_kernel_spmd

F32 = mybir.dt.float32
F32R = mybir.dt.float32r
BF16 = mybir.dt.bfloat16
OP = mybir.AluOpType
AF = mybir.ActivationFunctionType

B, C, HW = 16, 512, 1024  # batch, channels, H*W
NCORES = 8
BP = B // NCORES  # batches per core
NH, D = 8, 64  # heads, head dim
EPS = 1e-5
SCALE = D ** (-0.5)


def _emit(nc, x_d, out_d, qkvwT_d, projwT_d, gnw_d, gnb_d, qb_d, bvbc_d, pb_d, ones_d):
    with tile.TileContext(nc) as tc, ExitStack() as ctx:
        ec = ctx.enter_context
        consts = ec(tc.tile_pool(name="consts", bufs=1))
        xpool = ec(tc.tile_pool(name="xpool", bufs=4))     # x f32 [128,1024]
        xnpool = ec(tc.tile_pool(name="xnpool", bufs=8))  # xn f32r [128,1024]
        qkpool = ec(tc.tile_pool(name="qkpool", bufs=4))   # q,k bf16 [128,1024]
        vtpool = ec(tc.tile_pool(name="vtpool", bufs=8))   # vT bf16 [128,8,65]
        ppool = ec(tc.tile_pool(name="ppool", bufs=10))    # P bf16 [128,2,1024]
        opool = ec(tc.tile_pool(name="opool", bufs=4))     # o f32 [128,1024]
        outp = ec(tc.tile_pool(name="outp", bufs=2))       # final f32 [128,1024]
        small = ec(tc.tile_pool(name="small", bufs=8))     # stats
        rsp = ec(tc.tile_pool(name="rsp", bufs=2))         # recip-sum tiles
        tmpp = ec(tc.tile_pool(name="tmpp", bufs=2))       # odd-head bounce
        obpool = ec(tc.tile_pool(name="obpool", bufs=4))  # o bf16 for proj
        drp = ec(tc.tile_pool(name="drp", bufs=2, space="DRAM"))
        psmm = ec(tc.tile_pool(name="psmm", bufs=2, space="PSUM"))  # [128,1024]
        pssc = ec(tc.tile_pool(name="pssc", bufs=2, space="PSUM"))  # scores

        # ---- constants ----
        wqkv = []
        for ct in range(4):
            w_t = consts.tile([128, 1536], BF16, name=f"wqkv{ct}")
            nc.sync.dma_start(out=w_t, in_=qkvwT_d[ct * 128:(ct + 1) * 128, :])
            wqkv.append(w_t)
        wproj = []
        for ct in range(4):
            p_t = consts.tile([128, 512], BF16, name=f"wproj{ct}")
            nc.sync.dma_start(out=p_t, in_=projwT_d[ct * 128:(ct + 1) * 128, :])
            wproj.append(p_t)
        ones_blk = consts.tile([128, 128], BF16, name="ones_blk")
        nc.sync.dma_start(out=ones_blk, in_=ones_d[:, :])
        bvbc = consts.tile([128, 512], F32, name="bvbc")
        nc.sync.dma_start(out=bvbc, in_=bvbc_d[:, :])
        gnw_t, gnb_t, pb_t = [], [], []
        qb_t = []
        for ct in range(4):
            g_t = consts.tile([128, 1], F32, name=f"gnw{ct}")
            nc.sync.dma_start(out=g_t, in_=gnw_d[ct * 128:(ct + 1) * 128])
            gnw_t.append(g_t)
            b_t = consts.tile([128, 1], F32, name=f"gnb{ct}")
            nc.sync.dma_start(out=b_t, in_=gnb_d[ct * 128:(ct + 1) * 128])
            gnb_t.append(b_t)
            c_t = consts.tile([128, 1], F32, name=f"pb{ct}")
            nc.sync.dma_start(out=c_t, in_=pb_d[ct * 128:(ct + 1) * 128])
            pb_t.append(c_t)
        for mt in range(8):  # q,k biases only (first 1024 of qkv_b)
            q_t = consts.tile([128, 1], F32, name=f"qb{mt}")
            nc.sync.dma_start(out=q_t, in_=qb_d[mt * 128:(mt + 1) * 128])
            qb_t.append(q_t)
        eps_t = consts.tile([128, 1], F32, name="eps_t")
        nc.vector.memset(eps_t, EPS)


        # ---- GroupNorm for both batches (keeps ACT sqrt calls grouped) ----
        xn = {}  # (b, ct) -> normalized x tile [128, 1024] f32
        for b in range(BP):
            for ct in range(4):
                x_t = xpool.tile([128, 1024], F32, name="x_t")
                nc.sync.dma_start(out=x_t, in_=x_d[b, ct * 128:(ct + 1) * 128, :])
                st = small.tile([128, 2, 6], F32, name="st")
                nc.vector.bn_stats(out=st[:, 0, :], in_=x_t[:, 0:512])
                nc.vector.bn_stats(out=st[:, 1, :], in_=x_t[:, 512:1024])
                mv = small.tile([128, 2], F32, name="mv")
                nc.vector.bn_aggr(out=mv, in_=st)
                # mv := (row_mean, row_E2) with E2 = var + mean^2
                nc.vector.scalar_tensor_tensor(
                    out=mv[:, 1:2], in0=mv[:, 0:1], scalar=mv[:, 0:1],
                    in1=mv[:, 1:2], op0=OP.mult, op1=OP.add)
                mvh = small.tile([128, 2], BF16, name="mvh")
                nc.vector.tensor_copy(out=mvh, in_=mv)
                mvl = small.tile([128, 2], BF16, name="mvl")
                nc.vector.tensor_sub(out=mvl, in0=mv, in1=mvh)
                gstat = psmm.tile([128, 2], F32, tag="mm", name="gstat")
                nc.tensor.matmul(out=gstat, lhsT=ones_blk, rhs=mvh,
                                 start=True, stop=False)
                nc.tensor.matmul(out=gstat, lhsT=ones_blk, rhs=mvl,
                                 start=False, stop=True)
                gsb = small.tile([128, 2], F32, name="gsb")
                nc.vector.tensor_copy(out=gsb, in_=gstat)
                # group var = gE2 - gmean^2
                var_t = small.tile([128, 1], F32, name="var_t")
                nc.vector.tensor_mul(out=var_t, in0=gsb[:, 0:1], in1=gsb[:, 0:1])
                nc.vector.tensor_sub(out=var_t, in0=gsb[:, 1:2], in1=var_t)
                sd_t = small.tile([128, 1], F32, name="sd_t")
                nc.scalar.activation(out=sd_t, in_=var_t, func=AF.Sqrt,
                                     bias=eps_t[:, 0:1], scale=1.0)
                al_t = small.tile([128, 1], F32, name="al_t")
                nc.vector.reciprocal(out=al_t, in_=sd_t)
                # scale_c = alpha * w ; bias_c = b - mu * scale_c
                sc_t = small.tile([128, 1], F32, name="sc_t")
                nc.vector.tensor_mul(out=sc_t, in0=al_t, in1=gnw_t[ct])
                nmu_t = small.tile([128, 1], F32, name="nmu_t")
                nc.vector.tensor_scalar(out=nmu_t, in0=gsb[:, 0:1],
                                        scalar1=-1.0, scalar2=None, op0=OP.mult)
                bi_t = small.tile([128, 1], F32, name="bi_t")
                nc.vector.scalar_tensor_tensor(
                    out=bi_t, in0=nmu_t, scalar=sc_t, in1=gnb_t[ct],
                    op0=OP.mult, op1=OP.add)
                xn_t = xnpool.tile([128, 1024], BF16, name="xn_t")
                nc.vector.tensor_scalar(out=xn_t, in0=x_t, scalar1=sc_t,
                                        scalar2=bi_t, op0=OP.mult, op1=OP.add)
                xn[(b, ct)] = xn_t

        # ---- per-batch: qkv GEMM, attention, proj ----
        for b in range(BP):
            # q, k GEMM: psum[m 128, i 1024] = qkv_wT[:, m]^T @ xn ; m-tiles 0-7
            q_sb, k_sb = [], []
            for mt in range(8):
                ps = psmm.tile([128, 1024], F32, tag="mm", name="qk_ps")
                for ih in range(2):
                    for ct in range(4):
                        nc.tensor.matmul(
                            out=ps[:, ih * 512:(ih + 1) * 512],
                            lhsT=wqkv[ct][:, mt * 128:(mt + 1) * 128],
                            rhs=xn[(b, ct)][:, ih * 512:(ih + 1) * 512],
                            start=(ct == 0), stop=(ct == 3))
                dst = qkpool.tile([128, 1024], BF16,
                                  name="q_sb" if mt < 4 else "k_sb")
                nc.vector.tensor_scalar(out=dst, in0=ps, scalar1=qb_t[mt],
                                        scalar2=None, op0=OP.add)
                (q_sb if mt < 4 else k_sb).append(dst)
            # vT GEMM: psum[j 128, c 512] = xn[:, j]^T @ Wv^T ; j-tiles 0-7
            vt_sb = []
            for jt in range(8):
                ps = psmm.tile([128, 1024], F32, tag="mm", name="v_ps")
                for ct in range(4):
                    nc.tensor.matmul(
                        out=ps[:, 0:512],
                        lhsT=xn[(b, ct)][:, jt * 128:(jt + 1) * 128],
                        rhs=wqkv[ct][:, 1024:1536],
                        start=(ct == 0), stop=(ct == 3))
                vt_t = vtpool.tile([128, 8, 65], BF16, name="vt_t")
                nc.vector.memset(vt_t[:, :, 64:65], 1.0)
                nc.vector.tensor_tensor(
                    out=vt_t[:, :, 0:64],
                    in0=ps[:, 0:512].rearrange("p (h d) -> p h d", h=8),
                    in1=bvbc.rearrange("p (h d) -> p h d", h=8), op=OP.add)
                vt_sb.append(vt_t)

            # attention, head pairs, software-pipelined scores->AV
            def scores_pair(pr):
                ptiles = []
                for jt in range(8):
                    p_t = ppool.tile([128, 2, 1024], BF16, name="p_t")
                    for ih in range(2):
                        sc = pssc.tile([128, 1024], F32, tag="sc", name="sc_ps")
                        nc.tensor.matmul(
                            out=sc[:, 0:512],
                            lhsT=k_sb[pr][0:64, jt * 128:(jt + 1) * 128],
                            rhs=q_sb[pr][0:64, ih * 512:(ih + 1) * 512],
                            start=True, stop=True)
                        nc.tensor.matmul(
                            out=sc[:, 512:1024],
                            lhsT=k_sb[pr][64:128, jt * 128:(jt + 1) * 128],
                            rhs=q_sb[pr][64:128, ih * 512:(ih + 1) * 512],
                            start=True, stop=True)
                        nc.scalar.activation(
                            out=p_t[:, :, ih * 512:(ih + 1) * 512],
                            in_=sc.rearrange("p (h i) -> p h i", h=2),
                            func=AF.Exp, scale=SCALE)
                    ptiles.append(p_t)
                return ptiles

            def av_pair(pr, ptiles, o_t):
                for hs in range(2):
                    h = 2 * pr + hs
                    av = psmm.tile([65, 1024], F32, tag="mm", name="av_ps")
                    for ih in range(2):
                        for jt in range(8):
                            nc.tensor.matmul(
                                out=av[:, ih * 512:(ih + 1) * 512],
                                lhsT=vt_sb[jt][:, h, :],
                                rhs=ptiles[jt][:, hs, ih * 512:(ih + 1) * 512],
                                start=(jt == 0), stop=(jt == 7))
                    # softmax denominators sit in row 64; 1/sums -> broadcast
                    rs = rsp.tile([65, 1024], F32, name="rs")
                    nc.vector.reciprocal(out=rs[64:65, :], in_=av[64:65, :])
                    rs_dr = drp.tile([1024], F32, name="rs_dr")
                    nc.sync.dma_start(out=rs_dr, in_=rs[64:65, :])
                    rsb = rsp.tile([64, 1024], F32, name="rsb")
                    nc.sync.dma_start(
                        out=rsb,
                        in_=bass.AP(tensor=rs_dr.tensor, offset=rs_dr.offset,
                                    ap=[[0, 64]] + rs_dr.ap))
                    if hs == 0:
                        nc.vector.tensor_tensor(out=o_t[0:64, :], in0=av[0:64, :],
                                                in1=rsb, op=OP.mult)
                    else:
                        tmp = tmpp.tile([64, 1024], F32, name="tmp")
                        nc.vector.tensor_tensor(out=tmp, in0=av[0:64, :],
                                                in1=rsb, op=OP.mult)
                        nc.sync.dma_start(out=o_t[64:128, :], in_=tmp)

            o_sb = []
            prev = scores_pair(0)
            for pr in range(4):
                o_t = opool.tile([128, 1024], F32, name="o_t")
                o_sb.append(o_t)
                cur = prev
                if pr < 3:
                    prev = scores_pair(pr + 1)
                av_pair(pr, cur, o_t)

            # bf16 copies of o for the proj GEMM rhs
            ob_sb = []
            for kt in range(4):
                ob = obpool.tile([128, 1024], BF16, name="ob")
                nc.vector.tensor_copy(out=ob, in_=o_sb[kt])
                ob_sb.append(ob)
            # proj GEMM + residual + bias
            for ct in range(4):
                ps = psmm.tile([128, 1024], F32, tag="mm", name="pj_ps")
                for ih in range(2):
                    for kt in range(4):
                        nc.tensor.matmul(
                            out=ps[:, ih * 512:(ih + 1) * 512],
                            lhsT=wproj[kt][:, ct * 128:(ct + 1) * 128],
                            rhs=ob_sb[kt][:, ih * 512:(ih + 1) * 512],
                            start=(kt == 0), stop=(kt == 3))
                fin = outp.tile([128, 1024], F32, name="fin")
                nc.vector.scalar_tensor_tensor(
                    out=fin, in0=ps, scalar=pb_t[ct], in1=o_sb[ct],
                    op0=OP.add, op1=OP.add)
                nc.sync.dma_start(out=out_d[b, ct * 128:(ct + 1) * 128, :], in_=fin)


def build_nc():
    nc = bacc.Bacc(None, target_bir_lowering=False)
    x_d = nc.declare_dram_parameter("x", [BP, C, HW], F32, isOutput=False)
    qkvwT_d = nc.declare_dram_parameter("qkv_wT", [C, 3 * C], BF16, isOutput=False)
    projwT_d = nc.declare_dram_parameter("proj_wT", [C, C], BF16, isOutput=False)
    gnw_d = nc.declare_dram_parameter("gn_w", [C], F32, isOutput=False)
    gnb_d = nc.declare_dram_parameter("gn_b", [C], F32, isOutput=False)
    qb_d = nc.declare_dram_parameter("qkv_b_qk", [2 * C], F32, isOutput=False)
    bvbc_d = nc.declare_dram_parameter("qkv_b_v_bc", [128, 512], F32, isOutput=False)
    pb_d = nc.declare_dram_parameter("proj_b", [C], F32, isOutput=False)
    ones_d = nc.declare_dram_parameter("ones_blk", [128, 128], BF16, isOutput=False)
    out_d = nc.declare_dram_parameter("out", [BP, C, HW], F32, isOutput=True)
    _emit(nc, x_d, out_d, qkvwT_d, projwT_d, gnw_d, gnb_d, qb_d, bvbc_d, pb_d,
          ones_d)
    nc.compile()
    return nc


def make_in_maps(x, gn_w, gn_b, qkv_w, qkv_b, proj_w, proj_b):
    x = np.asarray(x, np.float32).reshape(B, C, HW)
    import ml_dtypes
    qkv_wT = np.ascontiguousarray(np.asarray(qkv_w, np.float32).T).astype(
        ml_dtypes.bfloat16)
    proj_wT = np.ascontiguousarray(np.asarray(proj_w, np.float32).T).astype(
        ml_dtypes.bfloat16)
    qkv_b = np.asarray(qkv_b, np.float32)
    bvbc = np.ascontiguousarray(
        np.broadcast_to(qkv_b[2 * C:3 * C][None, :], (128, C)))
    import ml_dtypes as _md
    ones_blk = np.zeros((128, 128), np.float32)
    for g in range(2):
        ones_blk[g * 64:(g + 1) * 64, g * 64:(g + 1) * 64] = 1.0 / 64.0
    ones_blk = ones_blk.astype(_md.bfloat16)
    common = {
        "qkv_wT": qkv_wT, "proj_wT": proj_wT,
        "gn_w": np.asarray(gn_w, np.float32),
        "gn_b": np.asarray(gn_b, np.float32),
        "qkv_b_qk": qkv_b[:2 * C].copy(),
        "qkv_b_v_bc": bvbc,
        "proj_b": np.asarray(proj_b, np.float32),
        "ones_blk": ones_blk,
    }
    return [dict(common, x=np.ascontiguousarray(x[c * BP:(c + 1) * BP]))
            for c in range(NCORES)]


_NC_CACHE = {}


def kernel(x, gn_w, gn_b, qkv_w, qkv_b, proj_w, proj_b):
    if "nc" not in _NC_CACHE:
        _NC_CACHE["nc"] = build_nc()
    nc = _NC_CACHE["nc"]
    in_maps = make_in_maps(x, gn_w, gn_b, qkv_w, qkv_b, proj_w, proj_b)
    res = run_bass_kernel_spmd(nc, in_maps, core_ids=list(range(NCORES)))
    out = np.concatenate([res.results[c]["out"] for c in range(NCORES)], axis=0)
    return out.reshape(B, C, 32, 32).astype(np.float32)


# revision 14
# speedup vs baseline: 35.3988x; 35.3988x over previous
"""Trainium2 Bass kernel for nn_AttentionBlock (GroupNorm + MHSA + proj residual).

Contract: kernel(**inputs) takes the FULL inputs from setup_inputs() and
returns the FULL output [16, 512, 32, 32]. Internally shards batch over the
8 NeuronCores (2 batches per core), runs one SPMD Bass program, gathers.
"""

import sys
from contextlib import ExitStack

sys.path.insert(0, "/opt/trn_rl_repo")

import numpy as np

import concourse.bass as bass
import concourse.bacc as bacc
import concourse.mybir as mybir
import concourse.tile as tile
from concourse.bass_utils import run_bass_kernel_spmd

F32 = mybir.dt.float32
F32R = mybir.dt.float32r
BF16 = mybir.dt.bfloat16
OP = mybir.AluOpType
AF = mybir.ActivationFunctionType

B, C, HW = 16, 512, 1024  # batch, channels, H*W
NCORES = 8
BP = B // NCORES  # batches per core
NH, D = 8, 64  # heads, head dim
EPS = 1e-5
SCALE = D ** (-0.5)


def _emit(nc, x_d, out_d, qkvwT_d, projwT_d, gnw_d, gnb_d, qb_d, bvbc_d, pb_d, ones_d, repeat=1):
    with tile.TileContext(nc) as tc, ExitStack() as ctx:
        ec = ctx.enter_context
        consts = ec(tc.tile_pool(name="consts", bufs=1))
        xpool = ec(tc.tile_pool(name="xpool", bufs=4))     # x f32 [128,1024]
        xnpool = ec(tc.tile_pool(name="xnpool", bufs=8))   # xn bf16 [128,1024]
        qkpool = ec(tc.tile_pool(name="qkpool", bufs=4))   # q,k bf16 [128,1024]
        vtpool = ec(tc.tile_pool(name="vtpool", bufs=8))   # vT bf16 [128,8,65]
        ppool = ec(tc.tile_pool(name="ppool", bufs=12))    # P bf16 [128,2,1024]
        opool = ec(tc.tile_pool(name="opool", bufs=4))     # o f32 [128,1024]
        obpool = ec(tc.tile_pool(name="obpool", bufs=4))   # o bf16 for proj
        outp = ec(tc.tile_pool(name="outp", bufs=2))       # final f32 [128,1024]
        small = ec(tc.tile_pool(name="small", bufs=8))     # stats
        rsp = ec(tc.tile_pool(name="rsp", bufs=3))         # recip-sum tiles
        tmpp = ec(tc.tile_pool(name="tmpp", bufs=3))       # odd-head bounce
        drp = ec(tc.tile_pool(name="drp", bufs=3, space="DRAM"))
        psmm = ec(tc.tile_pool(name="psmm", bufs=2, space="PSUM"))  # [128,512]
        pssc = ec(tc.tile_pool(name="pssc", bufs=2, space="PSUM"))  # [128,1024]
        psav = ec(tc.tile_pool(name="psav", bufs=2, space="PSUM"))  # [65,512]

        # ---- constants ----
        wqkv = []
        for ct in range(4):
            w_t = consts.tile([128, 1536], BF16, name=f"wqkv{ct}")
            nc.sync.dma_start(out=w_t, in_=qkvwT_d[ct * 128:(ct + 1) * 128, :])
            wqkv.append(w_t)
        wproj = []
        for ct in range(4):
            p_t = consts.tile([128, 512], BF16, name=f"wproj{ct}")
            nc.sync.dma_start(out=p_t, in_=projwT_d[ct * 128:(ct + 1) * 128, :])
            wproj.append(p_t)
        ones_blk = consts.tile([128, 128], BF16, name="ones_blk")
        nc.sync.dma_start(out=ones_blk, in_=ones_d[:, :])
        bvbc = consts.tile([128, 512], F32, name="bvbc")
        nc.sync.dma_start(out=bvbc, in_=bvbc_d[:, :])
        gnw_t, gnb_t, pb_t = [], [], []
        qb_t = []
        for ct in range(4):
            g_t = consts.tile([128, 1], F32, name=f"gnw{ct}")
            nc.sync.dma_start(out=g_t, in_=gnw_d[ct * 128:(ct + 1) * 128])
            gnw_t.append(g_t)
            b_t = consts.tile([128, 1], F32, name=f"gnb{ct}")
            nc.sync.dma_start(out=b_t, in_=gnb_d[ct * 128:(ct + 1) * 128])
            gnb_t.append(b_t)
            c_t = consts.tile([128, 1], F32, name=f"pb{ct}")
            nc.sync.dma_start(out=c_t, in_=pb_d[ct * 128:(ct + 1) * 128])
            pb_t.append(c_t)
        for mt in range(8):  # q,k biases only (first 1024 of qkv_b)
            q_t = consts.tile([128, 1], F32, name=f"qb{mt}")
            nc.sync.dma_start(out=q_t, in_=qb_d[mt * 128:(mt + 1) * 128])
            qb_t.append(q_t)
        eps_t = consts.tile([128, 1], F32, name="eps_t")
        nc.vector.memset(eps_t, EPS)

        # ---- GroupNorm (emitted per batch; see emission order below) ----
        xn = {}

        def gn_batch(b):
            for ct in range(4):
                x_t = xpool.tile([128, 1024], F32, name="x_t")
                nc.sync.dma_start(out=x_t, in_=x_d[b, ct * 128:(ct + 1) * 128, :])
                st = small.tile([128, 2, 6], F32, name="st")
                nc.vector.bn_stats(out=st[:, 0, :], in_=x_t[:, 0:512])
                nc.vector.bn_stats(out=st[:, 1, :], in_=x_t[:, 512:1024])
                mv = small.tile([128, 2], F32, name="mv")
                nc.vector.bn_aggr(out=mv, in_=st)
                # mv := (row_mean, row_E2)
                nc.vector.scalar_tensor_tensor(
                    out=mv[:, 1:2], in0=mv[:, 0:1], scalar=mv[:, 0:1],
                    in1=mv[:, 1:2], op0=OP.mult, op1=OP.add)
                # hi/lo bf16 split so the group-combine matmul stays exact
                mvh = small.tile([128, 2], BF16, name="mvh")
                nc.vector.tensor_copy(out=mvh, in_=mv)
                mvl = small.tile([128, 2], BF16, name="mvl")
                nc.vector.tensor_sub(out=mvl, in0=mv, in1=mvh)
                gstat = psmm.tile([128, 2], F32, tag="mm", name="gstat")
                nc.tensor.matmul(out=gstat, lhsT=ones_blk, rhs=mvh,
                                 start=True, stop=False)
                nc.tensor.matmul(out=gstat, lhsT=ones_blk, rhs=mvl,
                                 start=False, stop=True)
                gsb = small.tile([128, 2], F32, name="gsb")
                nc.vector.tensor_copy(out=gsb, in_=gstat)
                var_t = small.tile([128, 1], F32, name="var_t")
                nc.vector.tensor_mul(out=var_t, in0=gsb[:, 0:1], in1=gsb[:, 0:1])
                nc.vector.tensor_sub(out=var_t, in0=gsb[:, 1:2], in1=var_t)
                sd_t = small.tile([128, 1], F32, name="sd_t")
                nc.scalar.activation(out=sd_t, in_=var_t, func=AF.Sqrt,
                                     bias=eps_t[:, 0:1], scale=1.0)
                al_t = small.tile([128, 1], F32, name="al_t")
                nc.vector.reciprocal(out=al_t, in_=sd_t)
                sc_t = small.tile([128, 1], F32, name="sc_t")
                nc.vector.tensor_mul(out=sc_t, in0=al_t, in1=gnw_t[ct])
                nmu_t = small.tile([128, 1], F32, name="nmu_t")
                nc.vector.tensor_scalar(out=nmu_t, in0=gsb[:, 0:1],
                                        scalar1=-1.0, scalar2=None, op0=OP.mult)
                bi_t = small.tile([128, 1], F32, name="bi_t")
                nc.vector.scalar_tensor_tensor(
                    out=bi_t, in0=nmu_t, scalar=sc_t, in1=gnb_t[ct],
                    op0=OP.mult, op1=OP.add)
                xn_t = xnpool.tile([128, 1024], BF16, name="xn_t")
                nc.vector.tensor_scalar(out=xn_t, in0=x_t, scalar1=sc_t,
                                        scalar2=bi_t, op0=OP.mult, op1=OP.add)
                xn[(b, ct)] = xn_t

        def qkv_vt(b):
            q_sb, k_sb = [], []
            for mt in range(8):
                dst = qkpool.tile([128, 1024], BF16,
                                  name="q_sb" if mt < 4 else "k_sb")
                for ih in range(2):
                    ps = psmm.tile([128, 512], F32, tag="mm", name="qk_ps")
                    for ct in range(4):
                        nc.tensor.matmul(
                            out=ps,
                            lhsT=wqkv[ct][:, mt * 128:(mt + 1) * 128],
                            rhs=xn[(b, ct)][:, ih * 512:(ih + 1) * 512],
                            start=(ct == 0), stop=(ct == 3))
                    nc.vector.tensor_scalar(
                        out=dst[:, ih * 512:(ih + 1) * 512], in0=ps,
                        scalar1=qb_t[mt], scalar2=None, op0=OP.add)
                (q_sb if mt < 4 else k_sb).append(dst)
            vt_sb = []
            for jt in range(8):
                ps = psmm.tile([128, 512], F32, tag="mm", name="v_ps")
                for ct in range(4):
                    nc.tensor.matmul(
                        out=ps,
                        lhsT=xn[(b, ct)][:, jt * 128:(jt + 1) * 128],
                        rhs=wqkv[ct][:, 1024:1536],
                        start=(ct == 0), stop=(ct == 3))
                vt_t = vtpool.tile([128, 8, 65], BF16, name="vt_t")
                nc.vector.memset(vt_t[:, :, 64:65], 1.0)
                nc.vector.tensor_tensor(
                    out=vt_t[:, :, 0:64],
                    in0=ps.rearrange("p (h d) -> p h d", h=8),
                    in1=bvbc.rearrange("p (h d) -> p h d", h=8), op=OP.add)
                vt_sb.append(vt_t)
            return q_sb, k_sb, vt_sb

        def attn(b, q_sb, k_sb, vt_sb):
            def scores_pair(pr):
                ptiles = []
                for jt in range(8):
                    p_t = ppool.tile([128, 2, 1024], BF16, name="p_t")
                    for ih in range(2):
                        sc = pssc.tile([128, 1024], F32, tag="sc", name="sc_ps")
                        nc.tensor.matmul(
                            out=sc[:, 0:512],
                            lhsT=k_sb[pr][0:64, jt * 128:(jt + 1) * 128],
                            rhs=q_sb[pr][0:64, ih * 512:(ih + 1) * 512],
                            start=True, stop=True)
                        nc.tensor.matmul(
                            out=sc[:, 512:1024],
                            lhsT=k_sb[pr][64:128, jt * 128:(jt + 1) * 128],
                            rhs=q_sb[pr][64:128, ih * 512:(ih + 1) * 512],
                            start=True, stop=True)
                        nc.scalar.activation(
                            out=p_t[:, :, ih * 512:(ih + 1) * 512],
                            in_=sc.rearrange("p (h i) -> p h i", h=2),
                            func=AF.Exp, scale=SCALE)
                    ptiles.append(p_t)
                return ptiles

            def av_pair(pr, ptiles, o_t):
                for hs in range(2):
                    h = 2 * pr + hs
                    for ih in range(2):
                        av = psav.tile([65, 512], F32, name="av_ps")
                        for jt in range(8):
                            nc.tensor.matmul(
                                out=av,
                                lhsT=vt_sb[jt][:, h, :],
                                rhs=ptiles[jt][:, hs, ih * 512:(ih + 1) * 512],
                                start=(jt == 0), stop=(jt == 7))
                        rs = rsp.tile([65, 512], F32, name="rs")
                        nc.vector.reciprocal(out=rs[64:65, :], in_=av[64:65, :])
                        rs_dr = drp.tile([512], F32, name="rs_dr")
                        nc.sync.dma_start(out=rs_dr, in_=rs[64:65, :])
                        rsb = rsp.tile([64, 512], F32, name="rsb")
                        nc.sync.dma_start(
                            out=rsb,
                            in_=bass.AP(tensor=rs_dr.tensor, offset=rs_dr.offset,
                                        ap=[[0, 64]] + rs_dr.ap))
                        if hs == 0:
                            nc.vector.tensor_tensor(
                                out=o_t[0:64, ih * 512:(ih + 1) * 512],
                                in0=av[0:64, :], in1=rsb, op=OP.mult)
                        else:
                            tmp = tmpp.tile([64, 512], F32, name="tmp")
                            nc.vector.tensor_tensor(out=tmp, in0=av[0:64, :],
                                                    in1=rsb, op=OP.mult)
                            nc.sync.dma_start(
                                out=o_t[64:128, ih * 512:(ih + 1) * 512],
                                in_=tmp)

            o_sb = []
            prev = scores_pair(0)
            for pr in range(4):
                o_t = opool.tile([128, 1024], F32, name="o_t")
                o_sb.append(o_t)
                cur = prev
                if pr < 3:
                    prev = scores_pair(pr + 1)
                av_pair(pr, cur, o_t)
            return o_sb

        def proj(b, o_sb):
            # bf16 copies of o for the proj GEMM rhs
            ob_sb = []
            for kt in range(4):
                ob = obpool.tile([128, 1024], BF16, name="ob")
                nc.vector.tensor_copy(out=ob, in_=o_sb[kt])
                ob_sb.append(ob)
            # proj GEMM + residual + bias
            for ct in range(4):
                fin = outp.tile([128, 1024], F32, name="fin")
                for ih in range(2):
                    ps = psmm.tile([128, 512], F32, tag="mm", name="pj_ps")
                    for kt in range(4):
                        nc.tensor.matmul(
                            out=ps,
                            lhsT=wproj[kt][:, ct * 128:(ct + 1) * 128],
                            rhs=ob_sb[kt][:, ih * 512:(ih + 1) * 512],
                            start=(kt == 0), stop=(kt == 3))
                    nc.vector.scalar_tensor_tensor(
                        out=fin[:, ih * 512:(ih + 1) * 512], in0=ps,
                        scalar=pb_t[ct], in1=o_sb[ct][:, ih * 512:(ih + 1) * 512],
                        op0=OP.add, op1=OP.add)
                nc.sync.dma_start(out=out_d[b, ct * 128:(ct + 1) * 128, :], in_=fin)

        for rep in range(repeat):
            gn_batch(0)
            qkv0 = qkv_vt(0)
            gn_batch(1)
            o0 = attn(0, *qkv0)
            proj(0, o0)
            qkv1 = qkv_vt(1)
            o1 = attn(1, *qkv1)
            proj(1, o1)


def build_nc(repeat=1):
    nc = bacc.Bacc(None, target_bir_lowering=False)
    x_d = nc.declare_dram_parameter("x", [BP, C, HW], F32, isOutput=False)
    qkvwT_d = nc.declare_dram_parameter("qkv_wT", [C, 3 * C], BF16, isOutput=False)
    projwT_d = nc.declare_dram_parameter("proj_wT", [C, C], BF16, isOutput=False)
    gnw_d = nc.declare_dram_parameter("gn_w", [C], F32, isOutput=False)
    gnb_d = nc.declare_dram_parameter("gn_b", [C], F32, isOutput=False)
    qb_d = nc.declare_dram_parameter("qkv_b_qk", [2 * C], F32, isOutput=False)
    bvbc_d = nc.declare_dram_parameter("qkv_b_v_bc", [128, 512], F32, isOutput=False)
    pb_d = nc.declare_dram_parameter("proj_b", [C], F32, isOutput=False)
    ones_d = nc.declare_dram_parameter("ones_blk", [128, 128], BF16, isOutput=False)
    out_d = nc.declare_dram_parameter("out", [BP, C, HW], F32, isOutput=True)
    _emit(nc, x_d, out_d, qkvwT_d, projwT_d, gnw_d, gnb_d, qb_d, bvbc_d, pb_d,
          ones_d, repeat=repeat)
    nc.compile()
    return nc


def make_in_maps(x, gn_w, gn_b, qkv_w, qkv_b, proj_w, proj_b):
    x = np.asarray(x, np.float32).reshape(B, C, HW)
    import ml_dtypes
    qkv_wT = np.ascontiguousarray(np.asarray(qkv_w, np.float32).T).astype(
        ml_dtypes.bfloat16)
    proj_wT = np.ascontiguousarray(np.asarray(proj_w, np.float32).T).astype(
        ml_dtypes.bfloat16)
    qkv_b = np.asarray(qkv_b, np.float32)
    bvbc = np.ascontiguousarray(
        np.broadcast_to(qkv_b[2 * C:3 * C][None, :], (128, C)))
    import ml_dtypes as _md
    ones_blk = np.zeros((128, 128), np.float32)
    for g in range(2):
        ones_blk[g * 64:(g + 1) * 64, g * 64:(g + 1) * 64] = 1.0 / 64.0
    ones_blk = ones_blk.astype(_md.bfloat16)
    common = {
        "qkv_wT": qkv_wT, "proj_wT": proj_wT,
        "gn_w": np.asarray(gn_w, np.float32),
        "gn_b": np.asarray(gn_b, np.float32),
        "qkv_b_qk": qkv_b[:2 * C].copy(),
        "qkv_b_v_bc": bvbc,
        "proj_b": np.asarray(proj_b, np.float32),
        "ones_blk": ones_blk,
    }
    return [dict(common, x=np.ascontiguousarray(x[c * BP:(c + 1) * BP]))
            for c in range(NCORES)]


_NC_CACHE = {}


def kernel(x, gn_w, gn_b, qkv_w, qkv_b, proj_w, proj_b):
    if "nc" not in _NC_CACHE:
        _NC_CACHE["nc"] = build_nc()
    nc = _NC_CACHE["nc"]
    in_maps = make_in_maps(x, gn_w, gn_b, qkv_w, qkv_b, proj_w, proj_b)
    res = run_bass_kernel_spmd(nc, in_maps, core_ids=list(range(NCORES)))
    out = np.concatenate([res.results[c]["out"] for c in range(NCORES)], axis=0)
    return out.reshape(B, C, 32, 32).astype(np.float32)
